# revision 1
# baseline (speedup 1.0000x reference)
"""Linear attention (B=2, L=4096, DM=1024, H=16) on 8 trn2 NeuronCores.

Sharding: rows (B*L) split 8 ways -> each core owns 512 rows of each batch
(1024 rows total). Projections, feature map, denominators, V@S and the output
projection are all row-local. The only cross-core term is S = K^T Q per
(batch, head), reduced with TWO batch-split bf16 AllReduces of 128 KB each:
AR(b) fires as soon as batch b's q/k projections + S partials finish, so
batch 0's AllReduce overlaps batch 1's projections and both are fully hidden
behind the vT projection / first output half. ~149 us vs the 245.5 us
single-AR f32r baseline.

All matmuls run in bfloat16 (1 cycle/row at any free size on the PE, vs
float32r's 4x penalty below 256 -- and small-N matmuls dominate S/attnT);
psums accumulate in fp32.

Schedule/engine notes (matmul computes lhsT.T @ rhs, contracting over the
partition dim; every engine queue is in-order, so emission order is the
schedule):
 - xt/W live in single [128, 8*1024] SBUF tiles (col kc*1024+j holds
   element [kc*128+p, j]) filled by a few large DMAs -- every DMA costs
   ~625 ns on the serial HWDGE descriptor device, so ~50 DMAs total.
 - q/k: psum groups of 4 m-tiles; the ring of 8 banks keeps consecutive
   groups on disjoint banks so copyouts drain behind the next group's
   matmuls (no boundary stall, PE p-state stays at 2.4 GHz).
   ELU+1 = max(x,0)+exp(min(x,0)) on DVE+Act; outputs split per n-half.
 - S: head-pair packed blocks [k_h0|k_h1]^T [q_h0|q_h1] (half the matmuls,
   off-diagonal quarters wasted); diagonal quarters extracted by strided
   Act copies (Act's queue drains earlier than DVE's k-copyout backlog).
 - vT [d, m]: lhsT = Wv chunk; bias bv is per-PARTITION here, fused into
   the Act copyout (Identity + bias column) -- no bias matmuls.
 - denominators on DVE; transposed on PE (interleaved after vT t=3/t=5);
   reciprocal rows broadcast across partitions with PE selector matmuls
   (E_t.T @ recipT) -- gpsimd partition_broadcast needs partition-0/32/64/96
   sources and Pool.SEQ is monopolized by the collectives.
 - attnT[t] psum half j <- ccJ_j [64,64] S-slice.T @ vT (ccJ0/ccJ1 hold the
   reduced S at partitions 0-63/64-127 via one DMA each); one DVE mul
   applies the reciprocal during copyout.
 - out_dense: lhsT = attnT chunk; ccJ(1) DMAs are emitted between out(0)'s
   two store groups (SP queue ordering); the final psum groups taper
   (2,1,1) so the drain tail is one short copyout+store deep.
"""
import sys

sys.path.insert(0, "/opt/trn_rl_repo")
import numpy as np
import ml_dtypes

B, L, DM, H = 2, 4096, 1024, 16
D = DM // H  # 64
N_CORES = 8
ROWS = B * L // N_CORES  # 1024 rows per core
RPB = ROWS // B  # 512 rows per batch per core
NT = ROWS // 128  # 8 l-tiles per core (4 per batch)
KC = DM // 128  # 8 contraction chunks

_CACHE = {}


def _build():
    import concourse.bass as bass
    import concourse.mybir as mybir
    import concourse.tile as tile
    from concourse import bacc
    from concourse.masks import make_identity

    dt = mybir.dt
    f32, f32r, bf16 = dt.float32, dt.float32r, dt.bfloat16
    AFT = mybir.ActivationFunctionType

    nc = bacc.Bacc("TRN2", target_bir_lowering=False, debug=False,
                   num_devices=N_CORES)

    qT_d = nc.dram_tensor("qT", [DM, ROWS], bf16, kind="ExternalInput").ap()
    kT_d = nc.dram_tensor("kT", [DM, ROWS], bf16, kind="ExternalInput").ap()
    vT_d = nc.dram_tensor("vT", [DM, ROWS], bf16, kind="ExternalInput").ap()
    W_d = {w: nc.dram_tensor(w, [DM, DM], bf16, kind="ExternalInput").ap()
           for w in ("Wq", "Wk", "Wv", "Wo")}
    b_d = {b: nc.dram_tensor(b, [1, DM], bf16, kind="ExternalInput").ap()
           for b in ("bq", "bk", "bo")}
    bvT_d = nc.dram_tensor("bvT", [128, KC], f32, kind="ExternalInput").ap()
    E_d = nc.dram_tensor("Econst", [16, DM], bf16, kind="ExternalInput").ap()
    out_d = nc.dram_tensor("out", [ROWS, DM], f32, kind="ExternalOutput").ap()

    def big3(ap):
        # [DM, N] dram -> [128, KC, N] AP matching an [128, KC*N] SBUF tile
        return ap.rearrange("(kc p) n -> kc p n", p=128).transpose([1, 0, 2])

    with tile.TileContext(nc) as tc:
        with (
            tc.tile_pool(name="xt", bufs=1) as xt_pool,
            tc.tile_pool(name="w", bufs=1) as w_pool,
            tc.tile_pool(name="act", bufs=1) as act_pool,
            tc.tile_pool(name="tmp", bufs=3) as tmp_pool,
            tc.tile_pool(name="small", bufs=1) as small_pool,
            tc.tile_pool(name="ps", bufs=8, space="PSUM") as ps_pool,
            tc.tile_pool(name="dram", bufs=1, space="DRAM") as dram_pool,
        ):
            ones = small_pool.tile([1, 512], bf16, tag="ones", name="ones")
            nc.vector.memset(ones[:], 1.0)
            ident = small_pool.tile([128, 128], f32, tag="ident", name="ident")
            make_identity(nc, ident[:])
            Et = small_pool.tile([16, DM], bf16, tag="E", name="E")
            nc.sync.dma_start(Et[:], E_d)

            def load_bias(b, eng=None):
                t = small_pool.tile([1, DM], bf16, tag="bias", name="bias",
                                    bufs=2)
                # eng=gpsimd: the Pool DMA queue is idle before the
                # collectives, so early bias rows skip the serial HWDGE
                # backlog of the big xt/W loads
                (eng or nc.sync).dma_start(t[:], b_d[b])
                return t

            def load_big(dram_ap, pool, tag, nsplit=2, interleave=None):
                """interleave: second (dram_ap, pool, tag) loaded with its
                splits alternating with this one's (so the kc-ordered
                consumers of both tiles see chunks arrive in step)."""
                srcs = [(dram_ap, pool.tile([128, KC * 1024], bf16, tag=tag,
                                            name=tag))]
                if interleave is not None:
                    ap2, pool2, tag2 = interleave
                    srcs.append((ap2, pool2.tile([128, KC * 1024], bf16,
                                                 tag=tag2, name=tag2)))
                step = KC // nsplit
                for s in range(nsplit):
                    for ap, t in srcs:
                        src = big3(ap)
                        dst = t[:].rearrange("p (kc n) -> p kc n", kc=KC)
                        nc.sync.dma_start(dst[:, s * step:(s + 1) * step, :],
                                          src[:, s * step:(s + 1) * step, :])
                if interleave is not None:
                    return srcs[0][1], srcs[1][1]
                return srcs[0][1]

            # ================= q/k projections =================
            # psum groups of 4 m-tiles (one batch): group g+1's banks are
            # disjoint from group g's (ring of 8), so copyouts drain behind
            # the next group's matmuls -- no boundary stall, p-state hot.
            def proj_half(xt, wt, bias, outs, mh, first=False):
                """outs[m][l, d] = elu(X @ W + b)+1 for m-tiles of batch mh.
                first=True: the n=0 group takes its bias matmul LAST, so the
                kernel's very first matmuls depend only on the leading
                xt/W DMA chunks, not on the bias load."""
                ms = [mh * 4 + i for i in range(4)]
                for n in range(2):
                    bias_last = first and n == 0
                    psums = {m: ps_pool.tile([128, 512], f32, tag="pp",
                                             name="pp") for m in ms}
                    if not bias_last:
                        for m in ms:
                            nc.tensor.matmul(psums[m][:], ones[:1, :128],
                                             bias[:1, n * 512:(n + 1) * 512],
                                             start=True, stop=False)
                    for kc in range(KC):
                        for m in ms:
                            nc.tensor.matmul(
                                psums[m][:],
                                xt[:, kc * 1024 + m * 128:
                                   kc * 1024 + (m + 1) * 128],
                                wt[:, kc * 1024 + n * 512:
                                   kc * 1024 + (n + 1) * 512],
                                start=(bias_last and kc == 0),
                                stop=(not bias_last and kc == KC - 1))
                    if bias_last:
                        for m in ms:
                            nc.tensor.matmul(psums[m][:], ones[:1, :128],
                                             bias[:1, n * 512:(n + 1) * 512],
                                             start=False, stop=True)
                    for m in ms:
                        mn = tmp_pool.tile([128, 512], f32, tag="mn",
                                           name="mn", bufs=4)
                        ex = tmp_pool.tile([128, 512], f32, tag="ex",
                                           name="ex", bufs=4)
                        nc.vector.tensor_scalar_min(mn[:], psums[m][:], 0.0)
                        nc.scalar.activation(ex[:], mn[:], AFT.Exp)
                        nc.vector.scalar_tensor_tensor(
                            outs[m][n][:], psums[m][:], 0.0, ex[:],
                            op0=mybir.AluOpType.max,
                            op1=mybir.AluOpType.add)

            def s_partial(b):
                """S partial for batch b, packed as head-pair blocks:
                S_ps[g][:, i*128:+128] = [k_h0|k_h1]^T [q_h0|q_h1] for the
                pair t = g*4+i (h0=2t). Half the matmuls of per-head S; the
                off-diagonal quarters are waste, the diagonal quarters are
                extracted by strided copies in launch_ar."""
                S_ps = [ps_pool.tile([128, 512], f32, tag="pp", name="S_ps")
                        for _ in range(2)]
                for g in range(2):
                    for i in range(4):
                        c0 = 2 * i * 64
                        for lc in range(NT // B):
                            m = b * (NT // B) + lc
                            nc.tensor.matmul(
                                S_ps[g][:, i * 128:(i + 1) * 128],
                                k_t[m][g][:, c0:c0 + 128],
                                q_t[m][g][:, c0:c0 + 128],
                                start=(lc == 0), stop=(lc == NT // B - 1))
                return S_ps

            def launch_ar(b, S_ps):
                """Extract diagonal quarters (strided), fire the AllReduce.
                ccst rows 0:64 = even heads' S, rows 64:128 = odd heads'."""
                ccst = small_pool.tile([128, 512], bf16, tag=f"ccst{b}",
                                       name="ccst")
                for g in range(2):
                    for j in range(2):
                        src = S_ps[g][j * 64:(j + 1) * 64, :].rearrange(
                            "p (i c) -> p i c", i=4)[:, :, j * 64:(j + 1) * 64]
                        dst = ccst[j * 64:(j + 1) * 64,
                                   g * 256:(g + 1) * 256].rearrange(
                            "p (i d) -> p i d", i=4)
                        # Act engine: its queue drains earlier than DVE's
                        # (which is still chewing the k copyouts), so the
                        # AllReduce fires sooner
                        nc.scalar.activation(dst, src, AFT.Copy)
                cc_in = dram_pool.tile([128, 512], bf16, tag=f"ccin{b}",
                                       name="ccin")
                cc_out = dram_pool.tile([128, 512], bf16, tag=f"ccout{b}",
                                        name="ccout")
                nc.sync.dma_start(cc_in[:], ccst[:])
                nc.gpsimd.collective_compute(
                    "AllReduce", mybir.AluOpType.add,
                    replica_groups=[list(range(N_CORES))],
                    ins=[cc_in[:].opt()], outs=[cc_out[:].opt()])
                return cc_out

            bias_q = load_bias("bq", nc.gpsimd)
            bias_k = load_bias("bk", nc.gpsimd)
            xt_q, w_q = load_big(qT_d, xt_pool, "xtA", nsplit=8,
                                 interleave=(W_d["Wq"], w_pool, "wA"))
            xt_k, w_k = load_big(kT_d, xt_pool, "xtB", nsplit=8,
                                 interleave=(W_d["Wk"], w_pool, "wB"))

            # q/k feature tiles are split per n-half: the S matmuls for
            # head group g read only half g, and per-half tiles avoid a
            # false wait on the other half's ELU chain (dep tracking is
            # tile-granular)
            q_t = [[act_pool.tile([128, 512], bf16, tag=f"q{m}h{h}",
                                  name=f"q{m}h{h}") for h in range(2)]
                   for m in range(NT)]
            k_t = [[act_pool.tile([128, 512], bf16, tag=f"k{m}h{h}",
                                  name=f"k{m}h{h}") for h in range(2)]
                   for m in range(NT)]

            # batch 0: project, S partial, fire AllReduce 0 early
            proj_half(xt_q, w_q, bias_q, q_t, 0)
            proj_half(xt_k, w_k, bias_k, k_t, 0)
            cc0 = launch_ar(0, s_partial(0))

            # vT/out-proj loads: after the cc_in(0) DMA (SP is in-order; the
            # xt_v WAR wait on xtA would otherwise delay the collective),
            # before cc_in(1) so they issue as soon as the q tiles free.
            xt_v = load_big(vT_d, xt_pool, "xtA", nsplit=2)
            w_v = load_big(W_d["Wv"], w_pool, "wC", nsplit=2)
            bvT = small_pool.tile([128, KC], f32, tag="bvT", name="bvT")
            nc.sync.dma_start(bvT[:], bvT_d)
            w_o = load_big(W_d["Wo"], w_pool, "wA", nsplit=2)
            bias_o = load_bias("bo")

            # batch 1: project, S partial, AllReduce 1 (queues behind AR 0)
            proj_half(xt_q, w_q, bias_q, q_t, 1)
            proj_half(xt_k, w_k, bias_k, k_t, 1)
            cc1 = launch_ar(1, s_partial(1))

            # ---- denominators: den[m][l, h] = sum_d q*k on DVE (emitted
            # after the S staging so the AllReduces are not stuck behind
            # them in the in-order DVE queue) ----
            dens = []
            for m in range(NT):
                den = tmp_pool.tile([128, 16], f32, tag="den", name="den",
                                    bufs=NT)
                for half in range(2):
                    prod = tmp_pool.tile([128, 512], bf16, tag="prod",
                                         name="prod")
                    nc.vector.tensor_mul(
                        prod[:], q_t[m][half][:], k_t[m][half][:])
                    nc.vector.reduce_sum(
                        den[:, half * 8:(half + 1) * 8],
                        prod[:].rearrange("p (h d) -> p h d", h=8),
                        axis=mybir.AxisListType.X)
                dens.append(den)

            # ---- vT projection overlaps the AllReduces ----
            recipT = small_pool.tile([16, ROWS], f32, tag="recipT",
                                     name="recipT")
            recipT_r = small_pool.tile([16, ROWS], bf16, tag="recipTr",
                                       name="recipTr")
            rbs = [act_pool.tile([128, ROWS], bf16, tag=f"rb{t}",
                                 name=f"rb{t}") for t in range(KC)]

            def dent_half(b):
                # transpose dens -> recipT cols, reciprocal, bf16 copy
                for m in range(b * 4, b * 4 + 4):
                    dent = ps_pool.tile([16, 128], f32, tag="pp",
                                        name="dent")
                    nc.tensor.transpose(dent[:], dens[m][:], ident[:])
                    nc.vector.tensor_scalar_add(
                        recipT[:, m * 128:(m + 1) * 128], dent[:], 1e-6)
                sl = slice(b * 512, (b + 1) * 512)
                nc.vector.reciprocal(recipT[:, sl], recipT[:, sl])
                nc.vector.tensor_copy(recipT_r[:, sl], recipT[:, sl])

            def rb_half(b):
                # rb[t][p, b-half] = recip[2t + (p>=64)] via selector matmul
                for t in range(KC):
                    psr = ps_pool.tile([128, 512], f32, tag="pp", name="psr")
                    nc.tensor.matmul(psr[:], Et[:, t * 128:(t + 1) * 128],
                                     recipT_r[:, b * 512:(b + 1) * 512],
                                     start=True, stop=True)
                    nc.scalar.activation(rbs[t][:, b * 512:(b + 1) * 512],
                                         psr[:], AFT.Copy)

            vTs = [act_pool.tile([128, ROWS], bf16, tag=f"vt{t}",
                                 name=f"vt{t}")
                   for t in range(KC)]
            for t in range(KC):
                ps2 = [ps_pool.tile([128, 512], f32, tag="pp", name="pp")
                       for _ in range(2)]
                for kc in range(KC):
                    for n in range(2):
                        nc.tensor.matmul(
                            ps2[n][:],
                            w_v[:, kc * 1024 + t * 128:
                                kc * 1024 + (t + 1) * 128],
                            xt_v[:, kc * 1024 + n * 512:
                                 kc * 1024 + (n + 1) * 512],
                            start=(kc == 0), stop=(kc == KC - 1))
                if t == 3:
                    dent_half(0)
                elif t == 5:
                    dent_half(1)
                elif t == 6:
                    rb_half(0)
                for n in range(2):
                    # bias bv is per-partition here (rows = dm): fuse it
                    # into the copyout on the Act engine
                    nc.scalar.activation(
                        vTs[t][:, n * 512:(n + 1) * 512], ps2[n][:],
                        AFT.Identity, bias=bvT[:, t:t + 1])

            # ---- per batch: reload reduced S, attnT, output projection ----
            attnT = [act_pool.tile([128, ROWS], bf16, tag=f"at{t}",
                                   name=f"attnT{t}")
                     for t in range(KC)]

            def ccj_load(cc_out, b):
                ccJ0 = small_pool.tile([64, 512], bf16, tag=f"ccJ0{b}",
                                       name="ccJ0")
                ccJ1 = small_pool.tile([128, 512], bf16, tag=f"ccJ1{b}",
                                       name="ccJ1")
                nc.sync.dma_start(ccJ0[0:64, :], cc_out[0:64, :])
                nc.sync.dma_start(ccJ1[64:128, :], cc_out[64:128, :])
                return ccJ0, ccJ1

            def attn_half(ccJs, b):
                for t in range(KC):
                    ps = ps_pool.tile([128, 512], f32, tag="pp", name="pa")
                    for j in range(2):
                        col = (t // 4) * 256 + (t % 4) * 64
                        nc.tensor.matmul(
                            ps[j * 64:(j + 1) * 64, :],
                            ccJs[j][j * 64:(j + 1) * 64, col:col + 64],
                            vTs[t][j * 64:(j + 1) * 64,
                                   b * RPB:(b + 1) * RPB],
                            start=True, stop=True)
                    nc.vector.tensor_mul(
                        attnT[t][:, b * RPB:(b + 1) * RPB], ps[:],
                        rbs[t][:, b * RPB:(b + 1) * RPB])

            def out_half(mh, mid_hook=None, taper=False):
                base = mh * 4
                for n in range(2):
                    if n == 1 and mid_hook is not None:
                        mid_hook()
                    # taper the very last groups (2,1,1) so the drain tail
                    # is one short copyout+store deep instead of four
                    if taper and n == 1:
                        grps = [[base], [base + 1], [base + 2], [base + 3]]
                    else:
                        grps = [[base + i for i in range(4)]]
                    for ms in grps:
                        psums = {m: ps_pool.tile([128, 512], f32, tag="pp",
                                                 name="pp") for m in ms}
                        for m in ms:
                            nc.tensor.matmul(psums[m][:], ones[:1, :128],
                                             bias_o[:1,
                                                    n * 512:(n + 1) * 512],
                                             start=True, stop=False)
                        for kc in range(KC):
                            for m in ms:
                                nc.tensor.matmul(
                                    psums[m][:],
                                    attnT[kc][:, m * 128:(m + 1) * 128],
                                    w_o[:, kc * 1024 + n * 512:
                                        kc * 1024 + (n + 1) * 512],
                                    start=False, stop=(kc == KC - 1))
                        for m in ms:
                            ot = tmp_pool.tile([128, 512], f32,
                                               tag=("mn" if m % 2 else "ex"),
                                               name="ot", bufs=4)
                            if m % 2:
                                nc.scalar.activation(ot[:], psums[m][:],
                                                     AFT.Copy)
                            else:
                                nc.vector.tensor_copy(ot[:], psums[m][:])
                            nc.sync.dma_start(
                                out_d[m * 128:(m + 1) * 128,
                                      n * 512:(n + 1) * 512], ot[:])

            ccJs0 = ccj_load(cc0, 0)
            attn_half(ccJs0, 0)
            rb_half(1)
            # ccJ(1) DMAs are emitted between out(0)'s two store groups:
            # SP is in-order, so putting them after all out(0) stores would
            # delay them to ~the last store, stalling attnT(1); putting them
            # before would park SP on the AllReduce-1 semaphore and stall
            # the early stores instead.
            ccJs1 = []
            out_half(0, mid_hook=lambda: ccJs1.extend(ccj_load(cc1, 1)),
                     taper=True)
            attn_half(ccJs1, 1)
            out_half(1, taper=True)

    nc.compile()
    return nc


def _get_nc():
    if "nc" not in _CACHE:
        _CACHE["nc"] = _build()
    return _CACHE["nc"]


def _make_econst():
    E = np.zeros((16, DM), np.float32)
    for t in range(KC):
        E[2 * t, t * 128:t * 128 + 64] = 1.0
        E[2 * t + 1, t * 128 + 64:(t + 1) * 128] = 1.0
    return E


def kernel(query, key, value, Wq, bq, Wk, bk, Wv, bv, Wo, bo, **kw):
    from concourse.bass_utils import run_bass_kernel_spmd

    nc = _get_nc()
    bf = ml_dtypes.bfloat16
    query = np.asarray(query, dtype=np.float32)
    key = np.asarray(key, dtype=np.float32)
    value = np.asarray(value, dtype=np.float32)
    weights = {n: np.ascontiguousarray(np.asarray(w, np.float32).astype(bf))
               for n, w in (("Wq", Wq), ("Wk", Wk), ("Wv", Wv), ("Wo", Wo))}
    biases = {n: np.ascontiguousarray(
                  np.asarray(b, np.float32).reshape(1, DM).astype(bf))
              for n, b in (("bq", bq), ("bk", bk), ("bo", bo))}
    biases["bvT"] = np.ascontiguousarray(
        np.asarray(bv, np.float32).reshape(KC, 128).T)
    econst = _make_econst()

    in_maps = []
    for c in range(N_CORES):
        sl = slice(c * RPB, (c + 1) * RPB)
        m = {
            "qT": np.ascontiguousarray(
                np.concatenate([query[b, sl] for b in range(B)], 0).T
            ).astype(bf),
            "kT": np.ascontiguousarray(
                np.concatenate([key[b, sl] for b in range(B)], 0).T
            ).astype(bf),
            "vT": np.ascontiguousarray(
                np.concatenate([value[b, sl] for b in range(B)], 0).T
            ).astype(bf),
            "Econst": econst.astype(bf),
        }
        m.update(weights)
        m.update(biases)
        in_maps.append(m)

    res = run_bass_kernel_spmd(nc, in_maps, list(range(N_CORES)), **kw)
    out = np.empty((B, L, DM), np.float32)
    for c in range(N_CORES):
        o = np.asarray(res.results[c]["out"]).astype(np.float32)
        for b in range(B):
            out[b, c * RPB:(c + 1) * RPB] = o[b * RPB:(b + 1) * RPB]
    if kw:
        return out, res
    return out



# revision 25
# speedup vs baseline: 1.1177x; 1.1177x over previous
"""Linear attention (B=2, L=4096, DM=1024, H=16) on 8 trn2 NeuronCores.

Sharding: rows (B*L) split 8 ways; only cross-core term is S = K^T Q per
(batch, head), reduced with ONE 256KB bf16 AllReduce covering both batches
(the collective cost model is dominated by a 15us*1.875 fixed overhead, so
one AR beats the previous two batch-split ARs by ~28us of serialized
collective time).

Precision plan (rel-err gate 2e-2, bf16 baseline measures 4.3e-3):
 - q/k projections run in fp8 e4m3 with DoubleRow perf mode (0.5 cyc/row,
   2x bf16). Host applies balanced scaling x*a, W/a with a =
   sqrt(std(W)/std(x)) so both operands sit at the same std and W clears
   the e4m3 denormal floor. Numerically validated end-to-end ~1.5e-2.
 - v and out projections stay bf16: their fp8 error hits the output
   undamped (measured 4.3e-2 / 2.7e-2 -- fails the gate).
 - q/k features are written as fp8 (packed l-chunk pairs) so the S
   partials also run DoubleRow; the S error is damped by the L=4096
   averaging. Denominators come from the same fp8 features (consistent
   normalization, ~0.25% effect).

Schedule (every engine queue is in-order; emission order is the schedule):
 - ONE fp8 mega-load [qT|Wq|kT|Wk] (4MB; the pre-AR DMA floor) split in 8
   kc-chunks, consumed arrival-paced by q-batch0's 8 open psum groups;
   the bf16 [vT|Wv|Wo] load follows on the same SP queue.
 - bias via host-sent fp8 broadcast rows + [I|0] identity-pair DoubleRow
   matmul (256 cyc/group), emitted last in each group so the first
   matmuls depend only on the leading mega-load chunks.
 - S extraction (Act strided copies) -> cc_in DMA + AllReduce + ccJ
   reloads all on the Pool queue (SWDGE), which skips the busy SP/HWDGE
   path entirely.
 - vT projection + denominators + reciprocal broadcasts fill the AR
   window; attn + transposed out-projection form the tail.
 - out projection is TRANSPOSED (outT = Wo^T attnT): lhsT = Wo chunks,
   rhs = attnT tiles; bias bo becomes per-partition and fuses into the
   Act copyout; the [DM, ROWS] result is transposed on the host.
"""
import sys

sys.path.insert(0, "/opt/trn_rl_repo")
import numpy as np
import ml_dtypes

B, L, DM, H = 2, 4096, 1024, 16
D = DM // H  # 64
N_CORES = 8
ROWS = B * L // N_CORES  # 1024 rows per core
RPB = ROWS // B  # 512 rows per batch per core
NT = ROWS // 128  # 8 l-tiles per core (4 per batch)
KC = DM // 128  # 8 contraction chunks
KCP = KC // 2  # 4 fp8 DoubleRow chunk-pairs

_CACHE = {}


def _build():
    import concourse.bass as bass
    import concourse.mybir as mybir
    import concourse.tile as tile
    from concourse import bacc
    from concourse.masks import make_identity

    dt = mybir.dt
    f32, bf16, f8 = dt.float32, dt.bfloat16, dt.float8e4
    AFT = mybir.ActivationFunctionType
    DR = mybir.MatmulPerfMode.DoubleRow

    nc = bacc.Bacc("TRN2", target_bir_lowering=False, debug=False,
                   num_devices=N_CORES)

    W8 = 4 * 1024   # mega fp8 row: [qT | Wq | kT | Wk]
    W16 = 3 * 1024  # mega bf16 row: [vT | Wv | Wo]
    big8_d = nc.dram_tensor("big8", [DM, W8], f8, kind="ExternalInput").ap()
    big16_d = nc.dram_tensor("big16", [DM, W16], bf16,
                             kind="ExternalInput").ap()
    bq_d = nc.dram_tensor("bqbc", [128, 1536], f8, kind="ExternalInput").ap()
    bk_d = nc.dram_tensor("bkbc", [128, 1536], f8, kind="ExternalInput").ap()
    id8_d = nc.dram_tensor("id8", [128, 256], f8, kind="ExternalInput").ap()
    E_d = nc.dram_tensor("Econst", [16, DM], bf16, kind="ExternalInput").ap()
    bvT_d = nc.dram_tensor("bvT", [128, KC], f32, kind="ExternalInput").ap()
    boT_d = nc.dram_tensor("boT", [128, KC], f32, kind="ExternalInput").ap()
    out_d = nc.dram_tensor("out", [DM, ROWS], f32, kind="ExternalOutput").ap()

    def big3(ap, n):
        # [DM, n] dram -> [128, KC, n] AP matching a [128, KC*n] SBUF tile
        return ap.rearrange("(kc p) n -> kc p n", p=128).transpose([1, 0, 2])

    with tile.TileContext(nc) as tc:
        with (
            tc.tile_pool(name="xt", bufs=1) as xt_pool,
            tc.tile_pool(name="act", bufs=1) as act_pool,
            tc.tile_pool(name="tmp", bufs=3) as tmp_pool,
            tc.tile_pool(name="small", bufs=1) as small_pool,
            tc.tile_pool(name="ps", bufs=4, space="PSUM") as ps_pool,
            tc.tile_pool(name="dram", bufs=1, space="DRAM") as dram_pool,
        ):
            ident = small_pool.tile([128, 128], f32, tag="ident", name="ident")
            make_identity(nc, ident[:])
            neg1 = small_pool.tile([128, 1], f32, tag="neg1", name="neg1")
            nc.vector.memset(neg1[:], -1.0)

            # small loads ride the Pool SWDGE queue: no contention with the
            # big SP loads, available within ~2us
            id8 = small_pool.tile([128, 256], f8, tag="id8", name="id8")
            nc.gpsimd.dma_start(id8[:], id8_d)
            bias_q = small_pool.tile([128, 1536], f8, tag="bq", name="bq")
            nc.gpsimd.dma_start(bias_q[:], bq_d)
            bias_k = small_pool.tile([128, 1536], f8, tag="bk", name="bk")
            nc.gpsimd.dma_start(bias_k[:], bk_d)
            Et = small_pool.tile([16, DM], bf16, tag="E", name="E")
            nc.gpsimd.dma_start(Et[:], E_d)
            bvT = small_pool.tile([128, KC], f32, tag="bvT", name="bvT")
            nc.gpsimd.dma_start(bvT[:], bvT_d)
            boT = small_pool.tile([128, KC], f32, tag="boT", name="boT")
            nc.gpsimd.dma_start(boT[:], boT_d)

            big8t = xt_pool.tile([128, KC * W8], f8, tag="b8", name="b8")
            b8v = big8t[:].rearrange("p (kc n) -> p kc n", kc=KC)
            src8 = big3(big8_d, W8)
            for s in range(KC):
                nc.sync.dma_start(b8v[:, s:s + 1, :], src8[:, s:s + 1, :])
            big16t = xt_pool.tile([128, KC * W16], bf16, tag="b16",
                                  name="b16")
            b16v = big16t[:].rearrange("p (kc n) -> p kc n", kc=KC)
            src16 = big3(big16_d, W16)
            for s in range(2):
                nc.sync.dma_start(b16v[:, s * 4:(s + 1) * 4, :],
                                  src16[:, s * 4:(s + 1) * 4, :])

            # fp8 feature tiles packed [m0-h0 | m0-h1 | m1-h0 | m1-h1] per
            # l-tile pair: one [128,1024] copyout per l-tile, and the S
            # partials contract the l-chunk pair in one DoubleRow matmul
            # (mm-stride 1024 within the tile)
            qp = [act_pool.tile([128, 2048], f8, tag=f"q{mp}",
                                name=f"q{mp}") for mp in range(NT // 2)]
            kp = [act_pool.tile([128, 2048], f8, tag=f"k{mp}",
                                name=f"k{mp}") for mp in range(NT // 2)]

            def pair2(ap):
                return ap.rearrange("p (two c) -> p two c", two=2)

            def proj_batch(xoff, woff, bias_t, outs, mh, paced=False):
                """One batch of one projection: 4 double-bank psum tiles
                ([128,1024], one per m-tile), fp8 DoubleRow, chunk-pair-major
                when paced so the matmuls track DMA arrival; bias last via
                [I|0] pair matmul."""
                ms = [mh * 4 + i for i in range(4)]
                psums = {m: ps_pool.tile([128, 1024], f32, tag="pp2",
                                         name="pp2") for m in ms}

                def mm(c, n, m):
                    nc.tensor.matmul(
                        psums[m][:, n * 512:(n + 1) * 512],
                        b8v[:, 2 * c:2 * c + 2,
                            xoff + m * 128:xoff + (m + 1) * 128],
                        b8v[:, 2 * c:2 * c + 2,
                            woff + n * 512:woff + (n + 1) * 512],
                        start=(c == 0), stop=False, perf_mode=DR)

                def bias_mm(n, m):
                    nc.tensor.matmul(
                        psums[m][:, n * 512:(n + 1) * 512], pair2(id8[:]),
                        pair2(bias_t[:, n * 512:n * 512 + 1024]),
                        start=False, stop=True, perf_mode=DR)

                # The psum holds x+1 (the host bias rows carry +1), so
                # elu(x)+1 = max(x+1, exp(min(x,0))) = max(min(exp(x),1), x+1)
                # costs just TWO chained ops per l-tile: Act exp(in - 1) and
                # one DVE stt doing the min+max. (exp is monotonic, so
                # min(exp(x),1) == exp(min(x,0)); x ~ N(0,1) cannot
                # overflow exp in f32.)
                def copyout(m):
                    ex = tmp_pool.tile([128, 1024], f32, tag="ex",
                                       name="ex", bufs=4)
                    nc.scalar.activation(ex[:], psums[m][:], AFT.Exp,
                                         bias=neg1[:, 0:1])
                    # all stts on DVE: they read PSUM, which GPSIMD
                    # cannot access on real hardware
                    nc.vector.scalar_tensor_tensor(
                        outs[m // 2][:, (m % 2) * 1024:(m % 2) * 1024 + 1024],
                        ex[:], 1.0, psums[m][:],
                        op0=mybir.AluOpType.min,
                        op1=mybir.AluOpType.max)

                if paced:
                    for c in range(KCP):
                        for n in range(2):
                            for m in ms:
                                mm(c, n, m)
                    for m in ms:
                        for n in range(2):
                            bias_mm(n, m)
                        copyout(m)
                else:
                    for m in ms:
                        for n in range(2):
                            for c in range(KCP):
                                mm(c, n, m)
                            bias_mm(n, m)
                        copyout(m)

            def s_partial(b):
                """S partial for batch b, head-pair blocks as before but each
                matmul contracts an l-chunk PAIR via DoubleRow."""
                Sp = ps_pool.tile([128, 1024], f32, tag="pp2", name="S_ps")
                S_ps = [Sp[:, 0:512], Sp[:, 512:1024]]
                for g in range(2):
                    for i in range(4):
                        c0 = g * 512 + i * 128
                        for lcp in range(2):
                            mp = b * 2 + lcp
                            nc.tensor.matmul(
                                S_ps[g][:, i * 128:(i + 1) * 128],
                                pair2(kp[mp][:])[:, :, c0:c0 + 128],
                                pair2(qp[mp][:])[:, :, c0:c0 + 128],
                                start=(lcp == 0), stop=(lcp == 1),
                                perf_mode=DR)
                return S_ps

            ccst = small_pool.tile([128, 1024], bf16, tag="ccst", name="ccst")

            def extract(b, S_ps):
                # diagonal quarters -> ccst cols [b*512 : (b+1)*512]
                for g in range(2):
                    for j in range(2):
                        src = S_ps[g][j * 64:(j + 1) * 64, :].rearrange(
                            "p (i c) -> p i c", i=4)[:, :,
                                                     j * 64:(j + 1) * 64]
                        dst = ccst[j * 64:(j + 1) * 64,
                                   b * 512 + g * 256:
                                   b * 512 + (g + 1) * 256].rearrange(
                            "p (i d) -> p i d", i=4)
                        nc.scalar.activation(dst, src, AFT.Copy)

            # ---- batch 0 (paced against the mega-load), batch 1 ----
            proj_batch(0, 1024, bias_q, qp, 0, paced=True)
            proj_batch(2048, 3072, bias_k, kp, 0)
            S0 = s_partial(0)
            extract(0, S0)
            proj_batch(0, 1024, bias_q, qp, 1)
            proj_batch(2048, 3072, bias_k, kp, 1)
            S1 = s_partial(1)
            extract(1, S1)

            cc_in = dram_pool.tile([128, 1024], bf16, tag="ccin", name="ccin")
            cc_out = dram_pool.tile([128, 1024], bf16, tag="ccout",
                                    name="ccout")
            # batch-0 half ships as soon as its extract lands; only the
            # batch-1 half is on the critical path to the AR
            nc.gpsimd.dma_start(cc_in[:, 0:512], ccst[:, 0:512])
            nc.gpsimd.dma_start(cc_in[:, 512:1024], ccst[:, 512:1024])
            nc.gpsimd.collective_compute(
                "AllReduce", mybir.AluOpType.add,
                replica_groups=[list(range(N_CORES))],
                ins=[cc_in[:].opt()], outs=[cc_out[:].opt()])
            # ccJ reload: one DMA on the SP queue (idle between the big
            # loads and the out stores; HWDGE is free by then)
            ccJ = small_pool.tile([128, 1024], bf16, tag="ccJ", name="ccJ")
            nc.sync.dma_start(ccJ[:], cc_out[:])
            ccJs = [ccJ, ccJ]

            # ---- denominators on the Pool engine (free once the extracts
            # are done); they only need to be ready for dent_half during
            # the AR window ----
            dens = []
            for m in range(NT):
                den = tmp_pool.tile([128, 16], f32, tag="den", name="den",
                                    bufs=NT)
                for half in range(2):
                    prod = tmp_pool.tile([128, 512], bf16, tag="prod",
                                         name="prod")
                    sl = slice((m % 2) * 1024 + half * 512,
                               (m % 2) * 1024 + half * 512 + 512)
                    # muls on Pool (SBUF-only, legal there; Pool is idle
                    # during the feature chain); the X-axis reduce is
                    # DVE-only and wait-hinted past the feature chain (only
                    # needed by dent_half inside the AR window)
                    nc.gpsimd.tensor_mul(prod[:], qp[m // 2][:, sl],
                                         kp[m // 2][:, sl])
                    with tc.tile_wait_until(0.04):
                        nc.vector.reduce_sum(
                            den[:, half * 8:(half + 1) * 8],
                            prod[:].rearrange("p (h d) -> p h d", h=8),
                            axis=mybir.AxisListType.X)
                dens.append(den)

            # ---- vT projection + recip broadcasts fill the AR window ----
            recipT = small_pool.tile([16, ROWS], f32, tag="recipT",
                                     name="recipT")
            recipT_r = small_pool.tile([16, ROWS], bf16, tag="recipTr",
                                       name="recipTr")
            rbs = [act_pool.tile([128, ROWS], bf16, tag=f"rb{t}",
                                 name=f"rb{t}") for t in range(KC)]

            def dent_half(b):
                dentt = ps_pool.tile([128, 1024], f32, tag="pp2",
                                     name="dent")
                for i, m in enumerate(range(b * 4, b * 4 + 4)):
                    dent = dentt[0:16, i * 128:(i + 1) * 128]
                    nc.tensor.transpose(dent, dens[m][:], ident[:])
                    nc.vector.tensor_scalar_add(
                        recipT[:, m * 128:(m + 1) * 128], dent, 1e-6)
                sl = slice(b * 512, (b + 1) * 512)
                nc.vector.reciprocal(recipT[:, sl], recipT[:, sl])
                nc.vector.tensor_copy(recipT_r[:, sl], recipT[:, sl])

            def rb_half(b):
                for u in range(KC // 2):
                    psr = ps_pool.tile([128, 1024], f32, tag="pp2",
                                       name="psr")
                    for half in range(2):
                        t = 2 * u + half
                        nc.tensor.matmul(psr[:, half * 512:(half + 1) * 512],
                                         Et[:, t * 128:(t + 1) * 128],
                                         recipT_r[:, b * 512:(b + 1) * 512],
                                         start=True, stop=True)
                    for half in range(2):
                        t = 2 * u + half
                        nc.scalar.activation(
                            rbs[t][:, b * 512:(b + 1) * 512],
                            psr[:, half * 512:(half + 1) * 512], AFT.Copy)

            vTs = [act_pool.tile([128, ROWS], bf16, tag=f"vt{t}",
                                 name=f"vt{t}")
                   for t in range(KC)]
            for t in range(KC):
                ps2t = ps_pool.tile([128, 1024], f32, tag="pp2", name="pp")
                ps2 = [ps2t[:, 0:512], ps2t[:, 512:1024]]
                for kc in range(KC):
                    for n in range(2):
                        nc.tensor.matmul(
                            ps2[n][:],
                            big16t[:, kc * W16 + 1024 + t * 128:
                                   kc * W16 + 1024 + (t + 1) * 128],
                            big16t[:, kc * W16 + n * 512:
                                   kc * W16 + (n + 1) * 512],
                            start=(kc == 0), stop=(kc == KC - 1))
                if t == 3:
                    dent_half(0)
                elif t == 5:
                    dent_half(1)
                elif t == 6:
                    rb_half(0)
                elif t == 7:
                    rb_half(1)
                for n in range(2):
                    nc.scalar.activation(
                        vTs[t][:, n * 512:(n + 1) * 512], ps2[n],
                        AFT.Identity, bias=bvT[:, t:t + 1])

            # ---- tail: attn for both batches, transposed out projection ----
            attnT = [act_pool.tile([128, ROWS], bf16, tag=f"at{t}",
                                   name=f"attnT{t}")
                     for t in range(KC)]

            def attn_half(b):
                # all 16 matmuls back-to-back (no interleaved consumers):
                # any SEQ stall between singleton matmuls resets the PE
                # p-state ramp and the whole phase runs at 0.65 GHz
                pts = [ps_pool.tile([128, 1024], f32, tag="pp2", name="pa")
                       for _ in range(KC // 2)]
                pss = [pts[t // 2][:, (t % 2) * 512:(t % 2) * 512 + 512]
                       for t in range(KC)]
                for t in range(KC):
                    for j in range(2):
                        col = b * 512 + (t // 4) * 256 + (t % 4) * 64
                        nc.tensor.matmul(
                            pss[t][j * 64:(j + 1) * 64, :],
                            ccJs[j][j * 64:(j + 1) * 64, col:col + 64],
                            vTs[t][j * 64:(j + 1) * 64,
                                   b * RPB:(b + 1) * RPB],
                            start=True, stop=True)
                for t in range(KC):
                    nc.vector.tensor_mul(
                        attnT[t][:, b * RPB:(b + 1) * RPB], pss[t],
                        rbs[t][:, b * RPB:(b + 1) * RPB])

            def out_half(b, taper=False):
                pot = None
                for t in range(KC):
                    if t % 2 == 0:
                        pot = ps_pool.tile([128, 1024], f32, tag="pp2",
                                           name="po")
                    ps = pot[:, (t % 2) * 512:(t % 2) * 512 + 512]
                    c0 = b * 512
                    for kc in range(KC):
                        nc.tensor.matmul(
                            ps[:],
                            big16t[:, kc * W16 + 2048 + t * 128:
                                   kc * W16 + 2048 + (t + 1) * 128],
                            attnT[kc][:, c0:c0 + 512],
                            start=(kc == 0), stop=(kc == KC - 1))
                    ot = tmp_pool.tile([128, 512], f32,
                                       tag=("mn" if t % 2 else "ex"),
                                       name="ot", bufs=4)
                    if taper and t == KC - 1:
                        # split the very last copyout across Act+DVE and
                        # issue two half stores so the drain tail is short
                        nc.scalar.activation(ot[:, 0:256], ps[:, 0:256],
                                             AFT.Identity,
                                             bias=boT[:, t:t + 1])
                        nc.vector.tensor_scalar(
                            ot[:, 256:512], ps[:, 256:512],
                            boT[:, t:t + 1], None,
                            op0=mybir.AluOpType.add)
                        nc.sync.dma_start(
                            out_d[t * 128:(t + 1) * 128, c0:c0 + 256],
                            ot[:, 0:256])
                        nc.sync.dma_start(
                            out_d[t * 128:(t + 1) * 128,
                                  c0 + 256:c0 + 512],
                            ot[:, 256:512])
                        continue
                    # alternate copyout engines so the drain isn't
                    # serialized behind one engine's queue
                    if t % 2:
                        nc.scalar.activation(ot[:], ps[:], AFT.Identity,
                                             bias=boT[:, t:t + 1])
                    else:
                        nc.vector.tensor_scalar(
                            ot[:], ps[:], boT[:, t:t + 1], None,
                            op0=mybir.AluOpType.add)
                    nc.sync.dma_start(
                        out_d[t * 128:(t + 1) * 128, c0:c0 + 512], ot[:])

            # PE p-state warmers: the sim's clock ramp restarts after a
            # long idle (~13.4us of 0.65GHz before full speed), and the AR
            # window leaves the PE idle after vT/rb, which would put the
            # attn phase and the first out groups at low clock. Burn the
            # idle with f32 dummy matmuls (4 cyc/row -> 853ns each at full
            # clock) into a scratch psum: genuinely continuous PE work, no
            # cross-engine pacing chain needed.
            wps = ps_pool.tile([128, 1024], f32, tag="pp2", name="warm")
            NWARM = 16
            for i in range(NWARM):
                nc.tensor.matmul(wps[:, 0:512], recipT[0:16, 0:128],
                                 recipT[0:16, 0:512],
                                 start=(i == 0), stop=(i == NWARM - 1))

            attn_half(0)
            attn_half(1)
            out_half(0)
            out_half(1, taper=True)

    nc.compile()
    return nc


def _get_nc():
    if "nc" not in _CACHE:
        _CACHE["nc"] = _build()
    return _CACHE["nc"]


def _make_econst():
    E = np.zeros((16, DM), np.float32)
    for t in range(KC):
        E[2 * t, t * 128:t * 128 + 64] = 1.0
        E[2 * t + 1, t * 128 + 64:(t + 1) * 128] = 1.0
    return E


def kernel(query, key, value, Wq, bq, Wk, bk, Wv, bv, Wo, bo, **kw):
    from concourse.bass_utils import run_bass_kernel_spmd

    nc = _get_nc()
    F8 = ml_dtypes.float8_e4m3fn
    BF = ml_dtypes.bfloat16
    query = np.asarray(query, dtype=np.float32)
    key = np.asarray(key, dtype=np.float32)
    value = np.asarray(value, dtype=np.float32)
    Wq = np.asarray(Wq, np.float32)
    Wk = np.asarray(Wk, np.float32)
    aq = float(np.sqrt(Wq.std() / max(query.std(), 1e-30)))
    ak = float(np.sqrt(Wk.std() / max(key.std(), 1e-30)))
    Wq8 = (Wq / aq).astype(F8)
    Wk8 = (Wk / ak).astype(F8)
    Wv16 = np.asarray(Wv, np.float32).astype(BF)
    Wo16 = np.asarray(Wo, np.float32).astype(BF)

    def bias_bc(b):
        # +1 rides in the bias so the projection psum holds x+1 directly
        # (the ELU copyout identity needs it); fp8 represents 1.0 exactly
        t = np.zeros((128, 1536), np.float32)
        t[:, :1024] = np.asarray(b, np.float32).reshape(1, DM) + 1.0
        return t.astype(F8)

    bqbc = bias_bc(bq)
    bkbc = bias_bc(bk)
    id8 = np.zeros((128, 256), np.float32)
    id8[:, :128] = np.eye(128)
    id8 = id8.astype(F8)
    econst = _make_econst().astype(BF)
    bvT = np.ascontiguousarray(
        np.asarray(bv, np.float32).reshape(KC, 128).T)
    boT = np.ascontiguousarray(
        np.asarray(bo, np.float32).reshape(KC, 128).T)

    in_maps = []
    for c in range(N_CORES):
        sl = slice(c * RPB, (c + 1) * RPB)
        qT = np.concatenate([query[b, sl] for b in range(B)], 0).T
        kT = np.concatenate([key[b, sl] for b in range(B)], 0).T
        vT = np.concatenate([value[b, sl] for b in range(B)], 0).T
        big8 = np.concatenate(
            [(qT * aq).astype(F8), Wq8, (kT * ak).astype(F8), Wk8], axis=1)
        big16 = np.concatenate([vT.astype(BF), Wv16, Wo16], axis=1)
        m = {
            "big8": np.ascontiguousarray(big8),
            "big16": np.ascontiguousarray(big16),
            "bqbc": bqbc, "bkbc": bkbc, "id8": id8,
            "Econst": econst, "bvT": bvT, "boT": boT,
        }
        in_maps.append(m)

    res = run_bass_kernel_spmd(nc, in_maps, list(range(N_CORES)), **kw)
    out = np.empty((B, L, DM), np.float32)
    for c in range(N_CORES):
        o = np.asarray(res.results[c]["out"]).astype(np.float32)
        for b in range(B):
            out[b, c * RPB:(c + 1) * RPB] = o[:, b * RPB:(b + 1) * RPB].T
    if kw:
        return out, res
    return out


# revision 45
# speedup vs baseline: 1.1932x; 1.0675x over previous
"""Linear attention (B=2, L=4096, DM=1024, H=16) on 8 trn2 NeuronCores.

Sharding: rows (B*L) split 8 ways; only cross-core term is S = K^T Q per
(batch, head), reduced with ONE 256KB bf16 AllReduce covering both batches
(the collective cost model is dominated by a 15us*1.875 fixed overhead, so
one AR beats the previous two batch-split ARs by ~28us of serialized
collective time).

Precision plan (rel-err gate 2e-2, bf16 baseline measures 4.3e-3):
 - q/k projections run in fp8 e4m3 with DoubleRow perf mode (0.5 cyc/row,
   2x bf16). Host applies balanced scaling x*a, W/a with a =
   sqrt(std(W)/std(x)) so both operands sit at the same std and W clears
   the e4m3 denormal floor. Numerically validated end-to-end ~1.5e-2.
 - v and out projections stay bf16: their fp8 error hits the output
   undamped (measured 4.3e-2 / 2.7e-2 -- fails the gate).
 - q/k features are written as fp8 (packed l-chunk pairs) so the S
   partials also run DoubleRow; the S error is damped by the L=4096
   averaging. Denominators come from the same fp8 features (consistent
   normalization, ~0.25% effect).

Schedule (every engine queue is in-order; emission order is the schedule):
 - ONE fp8 mega-load [qT|Wq|kT|Wk] (4MB; the pre-AR DMA floor) split in 8
   kc-chunks, consumed arrival-paced by q-batch0's 8 open psum groups;
   the bf16 [vT|Wv|Wo] load follows on the same SP queue.
 - bias via host-sent fp8 broadcast rows + [I|0] identity-pair DoubleRow
   matmul (256 cyc/group), emitted last in each group so the first
   matmuls depend only on the leading mega-load chunks.
 - S extraction (Act strided copies) -> cc_in DMA + AllReduce + ccJ
   reloads all on the Pool queue (SWDGE), which skips the busy SP/HWDGE
   path entirely.
 - vT projection + denominators + reciprocal broadcasts fill the AR
   window; attn + transposed out-projection form the tail.
 - out projection is TRANSPOSED (outT = Wo^T attnT): lhsT = Wo chunks,
   rhs = attnT tiles; bias bo becomes per-partition and fuses into the
   Act copyout; the [DM, ROWS] result is transposed on the host.
"""
import sys

sys.path.insert(0, "/opt/trn_rl_repo")
import numpy as np
import ml_dtypes

B, L, DM, H = 2, 4096, 1024, 16
D = DM // H  # 64
N_CORES = 8
ROWS = B * L // N_CORES  # 1024 rows per core
RPB = ROWS // B  # 512 rows per batch per core
NT = ROWS // 128  # 8 l-tiles per core (4 per batch)
KC = DM // 128  # 8 contraction chunks
KCP = KC // 2  # 4 fp8 DoubleRow chunk-pairs

_CACHE = {}


def _build():
    import concourse.bass as bass
    import concourse.mybir as mybir
    import concourse.tile as tile
    from concourse import bacc
    from concourse.masks import make_identity

    dt = mybir.dt
    f32, bf16, f8 = dt.float32, dt.bfloat16, dt.float8e4
    AFT = mybir.ActivationFunctionType
    DR = mybir.MatmulPerfMode.DoubleRow

    nc = bacc.Bacc("TRN2", target_bir_lowering=False, debug=False,
                   num_devices=N_CORES)

    W8 = 4 * 1024   # mega fp8 row: [qT | Wq | kT | Wk]
    W16 = 3 * 1024  # mega bf16 row: [vT | Wv | Wo]
    big8_d = nc.dram_tensor("big8", [DM, W8], f8, kind="ExternalInput").ap()
    big16_d = nc.dram_tensor("big16", [DM, W16], bf16,
                             kind="ExternalInput").ap()
    bq_d = nc.dram_tensor("bqbc", [128, 1536], f8, kind="ExternalInput").ap()
    bk_d = nc.dram_tensor("bkbc", [128, 1536], f8, kind="ExternalInput").ap()
    id8_d = nc.dram_tensor("id8", [128, 256], f8, kind="ExternalInput").ap()
    E_d = nc.dram_tensor("Econst", [16, DM], bf16, kind="ExternalInput").ap()
    bvT_d = nc.dram_tensor("bvT", [128, KC], f32, kind="ExternalInput").ap()
    boT_d = nc.dram_tensor("boT", [128, KC], f32, kind="ExternalInput").ap()
    out_d = nc.dram_tensor("out", [DM, ROWS], f32, kind="ExternalOutput").ap()

    def big3(ap, n):
        # [DM, n] dram -> [128, KC, n] AP matching a [128, KC*n] SBUF tile
        return ap.rearrange("(kc p) n -> kc p n", p=128).transpose([1, 0, 2])

    with tile.TileContext(nc) as tc:
        with (
            tc.tile_pool(name="xt", bufs=1) as xt_pool,
            tc.tile_pool(name="act", bufs=1) as act_pool,
            tc.tile_pool(name="tmp", bufs=3) as tmp_pool,
            tc.tile_pool(name="small", bufs=1) as small_pool,
            tc.tile_pool(name="ps", bufs=4, space="PSUM") as ps_pool,
            tc.tile_pool(name="dram", bufs=1, space="DRAM") as dram_pool,
        ):
            ident = small_pool.tile([128, 128], f32, tag="ident", name="ident")
            make_identity(nc, ident[:])
            neg1 = small_pool.tile([128, 1], f32, tag="neg1", name="neg1")
            nc.vector.memset(neg1[:], -1.0)
            wexp = small_pool.tile([128, 1], f32, tag="wexp", name="wexp")
            nc.scalar.activation(wexp[:], neg1[:], AFT.Exp)

            # big8 in TWO splits: per-instruction overheads on the serial
            # DMA_ENGINES device make many small splits slower in aggregate
            # (8 splits deliver the last chunk ~16us; 2 splits ~12.5us),
            # and the arrival-paced matmul phase only does ~2.5us of work
            # anyway
            # id8/bias_q ahead of big8 (the q-batch0 bias matmul that
            # stops the first psum group needs them); big8 in 8 kc-splits
            # so the arrival-paced matmuls overlap the (pstate-low) ramp;
            # everything else behind it
            id8 = small_pool.tile([128, 256], f8, tag="id8", name="id8")
            nc.gpsimd.dma_start(id8[:], id8_d)
            bias_q = small_pool.tile([128, 1536], f8, tag="bq", name="bq")
            nc.gpsimd.dma_start(bias_q[:], bq_d)

            big8t = xt_pool.tile([128, KC * W8], f8, tag="b8", name="b8")
            b8v = big8t[:].rearrange("p (kc n) -> p kc n", kc=KC)
            src8 = big3(big8_d, W8)
            for s in range(KC):
                nc.sync.dma_start(b8v[:, s:s + 1, :], src8[:, s:s + 1, :])

            bias_k = small_pool.tile([128, 1536], f8, tag="bk", name="bk")
            nc.gpsimd.dma_start(bias_k[:], bk_d)
            Et = small_pool.tile([16, DM], bf16, tag="E", name="E")
            nc.gpsimd.dma_start(Et[:], E_d)
            bvT = small_pool.tile([128, KC], f32, tag="bvT", name="bvT")
            nc.gpsimd.dma_start(bvT[:], bvT_d)
            boT = small_pool.tile([128, KC], f32, tag="boT", name="boT")
            nc.gpsimd.dma_start(boT[:], boT_d)
            big16t = xt_pool.tile([128, KC * W16], bf16, tag="b16",
                                  name="b16")
            b16v = big16t[:].rearrange("p (kc n) -> p kc n", kc=KC)
            src16 = big3(big16_d, W16)
            for s in range(2):
                nc.sync.dma_start(b16v[:, s * 4:(s + 1) * 4, :],
                                  src16[:, s * 4:(s + 1) * 4, :])

            # start-warmers: begin the PE busy-run at ~0.5us so the
            # time-based clock ramp exits its low phase before the
            # post-load crunch
            w0 = small_pool.tile([1, 512], bf16, tag="w0", name="w0")
            nc.vector.memset(w0[:], 1.0)
            wps0 = ps_pool.tile([128, 1024], f32, tag="pp2", name="warm0")
            for i in range(7):
                nc.tensor.matmul(wps0[:, 0:512], w0[0:1, 0:128],
                                 w0[0:1, 0:512],
                                 start=(i == 0), stop=(i == 6))

            # fp8 feature tiles packed [m0-h0 | m0-h1 | m1-h0 | m1-h1] per
            # l-tile pair: one [128,1024] copyout per l-tile, and the S
            # partials contract the l-chunk pair in one DoubleRow matmul
            # (mm-stride 1024 within the tile)
            qp = [act_pool.tile([128, 2048], f8, tag=f"q{mp}",
                                name=f"q{mp}") for mp in range(NT // 2)]
            kp = [act_pool.tile([128, 2048], f8, tag=f"k{mp}",
                                name=f"k{mp}") for mp in range(NT // 2)]

            def pair2(ap):
                return ap.rearrange("p (two c) -> p two c", two=2)

            def proj_batch(xoff, woff, bias_t, outs, mh, paced=False):
                """One batch of one projection: 4 double-bank psum tiles
                ([128,1024], one per m-tile), fp8 DoubleRow, chunk-pair-major
                when paced so the matmuls track DMA arrival; bias last via
                [I|0] pair matmul."""
                ms = [mh * 4 + i for i in range(4)]
                psums = {m: ps_pool.tile([128, 1024], f32, tag="pp2",
                                         name="pp2") for m in ms}

                def mm(c, n, m):
                    nc.tensor.matmul(
                        psums[m][:, n * 512:(n + 1) * 512],
                        b8v[:, 2 * c:2 * c + 2,
                            xoff + m * 128:xoff + (m + 1) * 128],
                        b8v[:, 2 * c:2 * c + 2,
                            woff + n * 512:woff + (n + 1) * 512],
                        start=(c == 0), stop=False, perf_mode=DR)

                def bias_mm(n, m):
                    nc.tensor.matmul(
                        psums[m][:, n * 512:(n + 1) * 512], pair2(id8[:]),
                        pair2(bias_t[:, n * 512:n * 512 + 1024]),
                        start=False, stop=True, perf_mode=DR)

                # The psum holds x+1 (the host bias rows carry +1), so
                # elu(x)+1 = max(x+1, exp(min(x,0))) = max(min(exp(x),1), x+1)
                # costs just TWO chained ops per l-tile: Act exp(in - 1) and
                # one DVE stt doing the min+max. (exp is monotonic, so
                # min(exp(x),1) == exp(min(x,0)); x ~ N(0,1) cannot
                # overflow exp in f32.)
                def copyout(m):
                    ex = tmp_pool.tile([128, 1024], f32, tag="ex",
                                       name="ex", bufs=4)
                    nc.scalar.activation(ex[:], psums[m][:], AFT.Exp,
                                         bias=neg1[:, 0:1])
                    # all stts on DVE: they read PSUM, which GPSIMD
                    # cannot access on real hardware
                    nc.vector.scalar_tensor_tensor(
                        outs[m // 2][:, (m % 2) * 1024:(m % 2) * 1024 + 1024],
                        ex[:], 1.0, psums[m][:],
                        op0=mybir.AluOpType.min,
                        op1=mybir.AluOpType.max)

                if paced:
                    for c in range(KCP):
                        for n in range(2):
                            for m in ms:
                                mm(c, n, m)
                    for m in ms:
                        for n in range(2):
                            bias_mm(n, m)
                        copyout(m)
                else:
                    for m in ms:
                        for n in range(2):
                            for c in range(KCP):
                                mm(c, n, m)
                            bias_mm(n, m)
                        copyout(m)

            def s_partial(b):
                """S partial for batch b, head-pair blocks as before but each
                matmul contracts an l-chunk PAIR via DoubleRow."""
                Sp = ps_pool.tile([128, 1024], f32, tag="pp2", name="S_ps")
                S_ps = [Sp[:, 0:512], Sp[:, 512:1024]]
                for g in range(2):
                    for i in range(4):
                        c0 = g * 512 + i * 128
                        for lcp in range(2):
                            mp = b * 2 + lcp
                            nc.tensor.matmul(
                                S_ps[g][:, i * 128:(i + 1) * 128],
                                pair2(kp[mp][:])[:, :, c0:c0 + 128],
                                pair2(qp[mp][:])[:, :, c0:c0 + 128],
                                start=(lcp == 0), stop=(lcp == 1),
                                perf_mode=DR)
                return S_ps

            ccst = small_pool.tile([128, 1024], bf16, tag="ccst", name="ccst")

            def extract(b, S_ps):
                # diagonal quarters -> ccst cols [b*512 : (b+1)*512];
                # split across Act and DVE so the 4 copies drain in ~2 slots
                for g in range(2):
                    for j in range(2):
                        src = S_ps[g][j * 64:(j + 1) * 64, :].rearrange(
                            "p (i c) -> p i c", i=4)[:, :,
                                                     j * 64:(j + 1) * 64]
                        dst = ccst[j * 64:(j + 1) * 64,
                                   b * 512 + g * 256:
                                   b * 512 + (g + 1) * 256].rearrange(
                            "p (i d) -> p i d", i=4)
                        if b:
                            nc.vector.tensor_copy(dst, src)
                        else:
                            nc.scalar.activation(dst, src, AFT.Copy)

            # ---- batch 0 (paced against the mega-load), batch 1; S
            # partials LAST so the in-order PE queue never stalls the
            # projection supply on the feature-chain stts ----
            proj_batch(0, 1024, bias_q, qp, 0, paced=True)
            proj_batch(2048, 3072, bias_k, kp, 0)
            proj_batch(0, 1024, bias_q, qp, 1)
            proj_batch(2048, 3072, bias_k, kp, 1)
            S0 = s_partial(0)
            extract(0, S0)
            S1 = s_partial(1)
            extract(1, S1)

            cc_in = dram_pool.tile([128, 1024], bf16, tag="ccin", name="ccin")
            cc_rs = dram_pool.tile([16, 1024], bf16, tag="ccrs", name="ccrs")
            cc_out = dram_pool.tile([128, 1024], bf16, tag="ccout",
                                    name="ccout")
            # batch-0 half ships as soon as its extract lands; only the
            # batch-1 half is on the critical path to the collective
            nc.gpsimd.dma_start(cc_in[:, 0:512], ccst[:, 0:512])
            nc.sync.dma_start(cc_in[:, 512:1024], ccst[:, 512:1024])
            # ReduceScatter + AllGather instead of AllReduce: the cost
            # model charges AllReduce 1.875x its size-based time, while
            # RS+AG pay the (dominant) fixed overhead twice but no factor
            # -- net ~3us cheaper for this 256KB payload
            nc.gpsimd.collective_compute(
                "ReduceScatter", mybir.AluOpType.add,
                replica_groups=[list(range(N_CORES))],
                ins=[cc_in[:].opt()], outs=[cc_rs[:].opt()])
            nc.gpsimd.collective_compute(
                "AllGather", mybir.AluOpType.bypass,
                replica_groups=[list(range(N_CORES))],
                ins=[cc_rs[:].opt()], outs=[cc_out[:].opt()])
            # ccJ reload: halves on two independent DMA paths (SP HWDGE
            # and Pool SWDGE) so they land in parallel; the j=0 attn
            # matmuls only need rows 0:64
            ccJ = small_pool.tile([128, 1024], bf16, tag="ccJ", name="ccJ")
            nc.sync.dma_start(ccJ[0:64, :], cc_out[0:64, :])
            nc.gpsimd.dma_start(ccJ[64:128, :], cc_out[64:128, :])
            ccJs = [ccJ, ccJ]

            # ---- denominators on the Pool engine (free once the extracts
            # are done); they only need to be ready for dent_half during
            # the AR window ----
            dens = []
            for m in range(NT):
                den = tmp_pool.tile([128, 16], f32, tag="den", name="den",
                                    bufs=NT)
                for half in range(2):
                    prod = tmp_pool.tile([128, 512], bf16, tag="prod",
                                         name="prod")
                    sl = slice((m % 2) * 1024 + half * 512,
                               (m % 2) * 1024 + half * 512 + 512)
                    # muls on Pool (SBUF-only, legal there; Pool is idle
                    # during the feature chain); the X-axis reduce is
                    # DVE-only and wait-hinted past the feature chain (only
                    # needed by dent_half inside the AR window)
                    with tc.tile_wait_until(0.043):
                        nc.vector.tensor_mul(prod[:], qp[m // 2][:, sl],
                                             kp[m // 2][:, sl])
                        nc.vector.reduce_sum(
                            den[:, half * 8:(half + 1) * 8],
                            prod[:].rearrange("p (h d) -> p h d", h=8),
                            axis=mybir.AxisListType.X)
                dens.append(den)

            # ---- vT projection + recip broadcasts fill the AR window ----
            recipT = small_pool.tile([16, ROWS], f32, tag="recipT",
                                     name="recipT")
            recipT_r = small_pool.tile([16, ROWS], bf16, tag="recipTr",
                                       name="recipTr")
            # rb/attnT packed per t-pair [t-even cols 0:1024 | t-odd
            # 1024:2048] so one [128,1024] DVE mul covers a whole psum
            # pair-tile in the attn phase
            rbp = [act_pool.tile([128, 2 * ROWS], bf16, tag=f"rb{tp}",
                                 name=f"rb{tp}") for tp in range(KC // 2)]

            def dent_half(b):
                dentt = ps_pool.tile([128, 1024], f32, tag="pp2",
                                     name="dent")
                for i, m in enumerate(range(b * 4, b * 4 + 4)):
                    dent = dentt[0:16, i * 128:(i + 1) * 128]
                    nc.tensor.transpose(dent, dens[m][:], ident[:])
                    nc.vector.tensor_scalar_add(
                        recipT[:, m * 128:(m + 1) * 128], dent, 1e-6)
                sl = slice(b * 512, (b + 1) * 512)
                nc.vector.reciprocal(recipT[:, sl], recipT[:, sl])
                nc.vector.tensor_copy(recipT_r[:, sl], recipT[:, sl])

            def rb_half(b):
                for u in range(KC // 2):
                    psr = ps_pool.tile([128, 1024], f32, tag="pp2",
                                       name="psr")
                    for half in range(2):
                        t = 2 * u + half
                        nc.tensor.matmul(psr[:, half * 512:(half + 1) * 512],
                                         Et[:, t * 128:(t + 1) * 128],
                                         recipT_r[:, b * 512:(b + 1) * 512],
                                         start=True, stop=True)
                    for half in range(2):
                        t = 2 * u + half
                        nc.scalar.activation(
                            rbp[u][:, half * 1024 + b * 512:
                                   half * 1024 + (b + 1) * 512],
                            psr[:, half * 512:(half + 1) * 512], AFT.Copy)

            vTs = [act_pool.tile([128, ROWS], bf16, tag=f"vt{t}",
                                 name=f"vt{t}")
                   for t in range(KC)]
            for t in range(KC):
                ps2t = ps_pool.tile([128, 1024], f32, tag="pp2", name="pp")
                ps2 = [ps2t[:, 0:512], ps2t[:, 512:1024]]
                for kc in range(KC):
                    for n in range(2):
                        nc.tensor.matmul(
                            ps2[n][:],
                            big16t[:, kc * W16 + 1024 + t * 128:
                                   kc * W16 + 1024 + (t + 1) * 128],
                            big16t[:, kc * W16 + n * 512:
                                   kc * W16 + (n + 1) * 512],
                            start=(kc == 0), stop=(kc == KC - 1))
                for n in range(2):
                    nc.scalar.activation(
                        vTs[t][:, n * 512:(n + 1) * 512], ps2[n],
                        AFT.Identity, bias=bvT[:, t:t + 1])

            # ---- tail: attn for both batches, transposed out projection ----
            attnT = [act_pool.tile([128, 2 * ROWS], bf16, tag=f"at{tp}",
                                    name=f"attnT{tp}")
                     for tp in range(KC // 2)]

            def attn_half(b):
                # all 16 matmuls back-to-back (no interleaved consumers):
                # any SEQ stall between singleton matmuls resets the PE
                # p-state ramp and the whole phase runs at 0.65 GHz
                pts = [ps_pool.tile([128, 1024], f32, tag="pp2", name="pa")
                       for _ in range(KC // 2)]
                pss = [pts[t // 2][:, (t % 2) * 512:(t % 2) * 512 + 512]
                       for t in range(KC)]
                # matmuls per t as before; the mul runs once per pair
                for t in range(KC):
                    for j in range(2):
                        col = b * 512 + (t // 4) * 256 + (t % 4) * 64
                        nc.tensor.matmul(
                            pss[t][j * 64:(j + 1) * 64, :],
                            ccJs[j][j * 64:(j + 1) * 64, col:col + 64],
                            vTs[t][j * 64:(j + 1) * 64,
                                   b * RPB:(b + 1) * RPB],
                            start=True, stop=True)
                for tp in range(KC // 2):
                    def pv(ap):
                        return ap.rearrange("p (two c) -> p two c",
                                            two=2)[:, :,
                                                   b * RPB:(b + 1) * RPB]
                    nc.vector.tensor_mul(
                        pv(attnT[tp][:]), pts[tp][:].rearrange(
                            "p (two c) -> p two c", two=2),
                        pv(rbp[tp][:]))

            def out_half(b, taper=False):
                pot = None
                for t in range(KC):
                    if t % 2 == 0:
                        pot = ps_pool.tile([128, 1024], f32, tag="pp2",
                                           name="po")
                    ps = pot[:, (t % 2) * 512:(t % 2) * 512 + 512]
                    c0 = b * 512
                    if taper and t == KC - 1:
                        # taper in a FRESH psum tile: both sub-groups land
                        # on banks with no pending zero-region from this
                        # tile generation, so neither waits on a copyout
                        tap = ps_pool.tile([128, 1024], f32, tag="pp2",
                                           name="tap")
                        ps = tap[:, 0:512]
                        ot = tmp_pool.tile([128, 512], f32, tag="ex",
                                           name="ot", bufs=4)
                        for kc in range(KC):
                            nc.tensor.matmul(
                                ps[:, 0:384],
                                big16t[:, kc * W16 + 2048 + t * 128:
                                       kc * W16 + 2048 + (t + 1) * 128],
                                attnT[kc // 2][:, (kc % 2) * 1024 +
                                               c0:(kc % 2) * 1024 + c0 + 384],
                                start=(kc == 0), stop=(kc == KC - 1))
                        nc.scalar.activation(ot[:, 0:384], ps[:, 0:384],
                                             AFT.Identity,
                                             bias=boT[:, t:t + 1])
                        nc.sync.dma_start(
                            out_d[t * 128:(t + 1) * 128, c0:c0 + 384],
                            ot[:, 0:384])
                        # sliver in the fresh tile's other bank; store
                        # via SP (HWDGE beats SWDGE gen)
                        slv = tap[:, 512:640]
                        for kc in range(KC):
                            nc.tensor.matmul(
                                slv,
                                big16t[:, kc * W16 + 2048 + t * 128:
                                       kc * W16 + 2048 + (t + 1) * 128],
                                attnT[kc // 2][:, (kc % 2) * 1024 + c0 +
                                               384:(kc % 2) * 1024 + c0 + 512],
                                start=(kc == 0), stop=(kc == KC - 1))
                        nc.vector.tensor_scalar(
                            ot[:, 384:512], slv,
                            boT[:, t:t + 1], None,
                            op0=mybir.AluOpType.add)
                        nc.sync.dma_start(
                            out_d[t * 128:(t + 1) * 128,
                                  c0 + 384:c0 + 512],
                            ot[:, 384:512])
                        continue
                    for kc in range(KC):
                        nc.tensor.matmul(
                            ps[:],
                            big16t[:, kc * W16 + 2048 + t * 128:
                                   kc * W16 + 2048 + (t + 1) * 128],
                            attnT[kc // 2][:, (kc % 2) * 1024 +
                                           c0:(kc % 2) * 1024 + c0 + 512],
                            start=(kc == 0), stop=(kc == KC - 1))
                    ot = tmp_pool.tile([128, 512], f32,
                                       tag=("mn" if t % 2 else "ex"),
                                       name="ot", bufs=4)
                    # alternate copyout engines so the drain isn't
                    # serialized behind one engine's queue
                    if t % 2:
                        nc.scalar.activation(ot[:], ps[:], AFT.Identity,
                                             bias=boT[:, t:t + 1])
                    else:
                        nc.vector.tensor_scalar(
                            ot[:], ps[:], boT[:, t:t + 1], None,
                            op0=mybir.AluOpType.add)
                    nc.sync.dma_start(
                        out_d[t * 128:(t + 1) * 128, c0:c0 + 512], ot[:])

            dent_half(0)
            dent_half(1)
            rb_half(0)
            rb_half(1)

            # PE p-state warmers: the sim's clock ramp restarts after a
            # long idle (~13.4us of 0.65GHz before full speed), and the AR
            # window leaves the PE idle after vT/rb, which would put the
            # attn phase and the first out groups at low clock. Burn the
            # idle with f32 dummy matmuls (4 cyc/row -> 853ns each at full
            # clock) into a scratch psum: genuinely continuous PE work, no
            # cross-engine pacing chain needed.
            wps = ps_pool.tile([128, 1024], f32, tag="pp2", name="warm")
            NWARM = 9
            for i in range(NWARM):
                nc.tensor.matmul(wps[:, 0:512], recipT[0:16, 0:128],
                                 recipT[0:16, 0:512],
                                 start=(i == 0), stop=(i == NWARM - 1))

            attn_half(0)
            attn_half(1)
            out_half(0)
            out_half(1, taper=True)

    nc.compile()
    return nc


def _get_nc():
    if "nc" not in _CACHE:
        _CACHE["nc"] = _build()
    return _CACHE["nc"]


def _make_econst():
    E = np.zeros((16, DM), np.float32)
    for t in range(KC):
        E[2 * t, t * 128:t * 128 + 64] = 1.0
        E[2 * t + 1, t * 128 + 64:(t + 1) * 128] = 1.0
    return E


def kernel(query, key, value, Wq, bq, Wk, bk, Wv, bv, Wo, bo, **kw):
    from concourse.bass_utils import run_bass_kernel_spmd

    nc = _get_nc()
    F8 = ml_dtypes.float8_e4m3fn
    BF = ml_dtypes.bfloat16
    query = np.asarray(query, dtype=np.float32)
    key = np.asarray(key, dtype=np.float32)
    value = np.asarray(value, dtype=np.float32)
    Wq = np.asarray(Wq, np.float32)
    Wk = np.asarray(Wk, np.float32)
    aq = float(np.sqrt(Wq.std() / max(query.std(), 1e-30)))
    ak = float(np.sqrt(Wk.std() / max(key.std(), 1e-30)))
    Wq8 = (Wq / aq).astype(F8)
    Wk8 = (Wk / ak).astype(F8)
    Wv16 = np.asarray(Wv, np.float32).astype(BF)
    Wo16 = np.asarray(Wo, np.float32).astype(BF)

    def bias_bc(b):
        # +1 rides in the bias so the projection psum holds x+1 directly
        # (the ELU copyout identity needs it); fp8 represents 1.0 exactly
        t = np.zeros((128, 1536), np.float32)
        t[:, :1024] = np.asarray(b, np.float32).reshape(1, DM) + 1.0
        return t.astype(F8)

    bqbc = bias_bc(bq)
    bkbc = bias_bc(bk)
    id8 = np.zeros((128, 256), np.float32)
    id8[:, :128] = np.eye(128)
    id8 = id8.astype(F8)
    econst = _make_econst().astype(BF)
    bvT = np.ascontiguousarray(
        np.asarray(bv, np.float32).reshape(KC, 128).T)
    boT = np.ascontiguousarray(
        np.asarray(bo, np.float32).reshape(KC, 128).T)

    in_maps = []
    for c in range(N_CORES):
        sl = slice(c * RPB, (c + 1) * RPB)
        qT = np.concatenate([query[b, sl] for b in range(B)], 0).T
        kT = np.concatenate([key[b, sl] for b in range(B)], 0).T
        vT = np.concatenate([value[b, sl] for b in range(B)], 0).T
        big8 = np.concatenate(
            [(qT * aq).astype(F8), Wq8, (kT * ak).astype(F8), Wk8], axis=1)
        big16 = np.concatenate([vT.astype(BF), Wv16, Wo16], axis=1)
        m = {
            "big8": np.ascontiguousarray(big8),
            "big16": np.ascontiguousarray(big16),
            "bqbc": bqbc, "bkbc": bkbc, "id8": id8,
            "Econst": econst, "bvT": bvT, "boT": boT,
        }
        in_maps.append(m)

    res = run_bass_kernel_spmd(nc, in_maps, list(range(N_CORES)), **kw)
    out = np.empty((B, L, DM), np.float32)
    for c in range(N_CORES):
        o = np.asarray(res.results[c]["out"]).astype(np.float32)
        for b in range(B):
            out[b, c * RPB:(c + 1) * RPB] = o[:, b * RPB:(b + 1) * RPB].T
    if kw:
        return out, res
    return out
